# revision 6
# baseline (speedup 1.0000x reference)
"""Trainium2 Bass kernel for nn_DWTExtractor.

Computes, for each single-channel 1024x1024 image, 6 output channels
(3 Haar DWT2 details + 3 Coif1 DWT2 details bilinearly resized to 512x512).

Everything is linear and separable, so each channel is
    chan = RowM @ img @ ColM^T
with RowM/ColM in {Hlo, Hhi, RClo, RChi} (all [512, 1024] banded operators;
RC* fold the coif1 DWT with the jax.image.resize 514->512 linear+antialias
matrix). Both passes run on the TensorEngine with the *data* as the
stationary operand (lhsT), so each pass's output lands in PSUM already
transposed for the next pass - no transpose instructions at all:

  pass 1: T2[op][w, n] = sum_h X[h, w] * Op[n, h]
          lhsT = X[h-window, w-window] (128x128), rhs = packed band matrix
  pass 2: chan[m, n]   = sum_w T2[op][w, m] * Col[n, w]
          lhsT = T2[w-window, m-slice], rhs = band slice

The image axes are covered by 9 overlapping 128-wide windows (stride 114)
so that every output column's 12-tap support lies inside a single window;
each window writes a disjoint column slice (singleton PSUM groups, no
cross-window accumulation). Pass-1 packs all 4 operators' bands into one
[128, 228] rhs per window -> one matmul per (w-window, h-window).

Sharding: pure data parallel, 32 images -> 8 cores x 4 images.
"""

import sys

sys.path.insert(0, "/opt/trn_rl_repo")

from contextlib import ExitStack

import numpy as np

import concourse.bass as bass
from concourse import bacc
import concourse.mybir as mybir
import concourse.tile as tile
from concourse.bass_utils import run_bass_kernel_spmd

# ---------------------------------------------------------------------------
# Host-side operator construction (pure numpy, float64)
# ---------------------------------------------------------------------------

_c = np.array([-0.01565572813546454, -0.0727326195128539, 0.38486484686420286,
               0.8525720202122554, 0.3378976624578092, -0.0727326195128539])
HAAR_LO = np.array([0.7071067811865476, 0.7071067811865476])
HAAR_HI = np.array([-0.7071067811865476, 0.7071067811865476])
COIF1_LO = _c.copy()
COIF1_HI = ((-1.0) ** (np.arange(6) + 1)) * _c[::-1]

H = 1024
NT = 9            # overlapping 128-row windows, stride 114
SLOT = 57         # output columns assigned per window (57 * 9 = 513)
ROW_START = [min(max(114 * t - 6, 0), H - 128) for t in range(NT)]
N_CORES = 8
B_TOTAL = 32
BPC = B_TOTAL // N_CORES

# channel -> (row op index, col op index); ops are [Hlo, Hhi, RClo, RChi]
CHAN_OPS = [(1, 0), (0, 1), (1, 1), (3, 2), (2, 3), (3, 3)]
# channel emission order: channels sharing the same pass-1 tensor adjacent
CH_ORDER = [0, 2, 1, 3, 5, 4]

DT = mybir.dt.float16
NPDT = np.float16


def _dwt1d_np(x, filt):
    L = len(filt)
    n = x.shape[-1]
    xp = np.pad(x, [(0, 0)] * (x.ndim - 1) + [(L - 1, L - 1)], mode="symmetric")
    out_len = (n + L - 1) // 2
    fr = filt[::-1]
    y = np.zeros(x.shape[:-1] + (out_len,), dtype=x.dtype)
    for j in range(L):
        y = y + fr[j] * xp[..., 1 + j:1 + j + 2 * out_len:2]
    return y


def _dwt_matrix(n, filt):
    eye = np.eye(n, dtype=np.float64)
    return _dwt1d_np(eye, np.asarray(filt, np.float64)).T.copy()


def _resize_matrix(in_size, out_size):
    """Replicates jax.image.resize(method='linear', antialias=True)."""
    scale = out_size / in_size
    inv_scale = 1.0 / scale
    kernel_scale = max(inv_scale, 1.0)
    sample_f = (np.arange(out_size, dtype=np.float64) + 0.5) * inv_scale - 0.5
    x = np.abs(sample_f[None, :]
               - np.arange(in_size, dtype=np.float64)[:, None]) / kernel_scale
    w = np.maximum(0.0, 1.0 - x)
    total = w.sum(axis=0, keepdims=True)
    w = np.where(np.abs(total) > 1000.0 * np.finfo(np.float32).eps,
                 w / np.where(total != 0, total, 1), 0.0)
    w = np.where(((sample_f >= -0.5) & (sample_f <= in_size - 0.5))[None, :],
                 w, 0.0)
    return w.T.copy()


def build_ops():
    Hlo = _dwt_matrix(H, HAAR_LO)
    Hhi = _dwt_matrix(H, HAAR_HI)
    Clo = _dwt_matrix(H, COIF1_LO)
    Chi = _dwt_matrix(H, COIF1_HI)
    R = _resize_matrix(514, 512)
    return [Hlo, Hhi, R @ Clo, R @ Chi]


def assigned(t):
    return SLOT * t, min(SLOT * (t + 1), 512)


def build_bands(ops):
    """wmat [NT, 128, 4*SLOT]: per-window packed band matrices."""
    wmat = np.zeros((NT, 128, 4 * SLOT), np.float64)
    for t in range(NT):
        rs = ROW_START[t]
        n0, n1 = assigned(t)
        for f in range(4):
            full = ops[f][n0:n1]
            mask = np.zeros(H, bool)
            mask[rs:rs + 128] = True
            assert np.abs(full[:, ~mask]).max() == 0.0, (t, f)
            wmat[t, :, f * SLOT:f * SLOT + (n1 - n0)] = full[:, rs:rs + 128].T
    return wmat


# ---------------------------------------------------------------------------
# Bass kernel
# ---------------------------------------------------------------------------

def build_nc(bpc=BPC):
    nc = bacc.Bacc("TRN2", num_swdge_queues=4)
    x = nc.dram_tensor("x", [bpc, H, H], mybir.dt.float32, kind="ExternalInput")
    w = nc.dram_tensor("w", [NT, 128, 4 * SLOT], DT, kind="ExternalInput")
    y = nc.dram_tensor("y", [bpc, 6, 512, 512], mybir.dt.float32,
                       kind="ExternalOutput")

    with tile.TileContext(nc) as tc, ExitStack() as ctx:
        const = ctx.enter_context(tc.tile_pool(name="const", bufs=1))
        xstage = ctx.enter_context(tc.tile_pool(name="xstage", bufs=3))
        xhalf = ctx.enter_context(tc.tile_pool(name="xhalf", bufs=2))
        t2p = ctx.enter_context(tc.tile_pool(name="t2p", bufs=2))
        outs = ctx.enter_context(tc.tile_pool(name="outs", bufs=8))

        wt_tiles = []
        for t in range(NT):
            wt_t = const.tile([128, 4 * SLOT], DT, name=f"w{t}", tag=f"w{t}")
            nc.gpsimd.dma_start(wt_t[:], w[t])
            wt_tiles.append(wt_t)

        for i in range(bpc):
            # ---- load + fp16 cast ----
            xtiles = []
            for t in range(NT):
                xs_t = xstage.tile([128, H], mybir.dt.float32,
                                   name="xs", tag="xs")
                nc.gpsimd.dma_start(
                    xs_t[:], x[i, ROW_START[t]:ROW_START[t] + 128, :])
                xb_t = xhalf.tile([128, H], DT, name=f"xb{t}", tag=f"xb{t}")
                nc.gpsimd.tensor_copy(xb_t[:], xs_t[:])
                xtiles.append(xb_t)

            # ---- pass 1 ----
            t2 = {}
            with tc.tile_pool(name=f"p1_{i}", bufs=2, space="PSUM") as p1:
                for wt in range(NT):
                    ws = ROW_START[wt]
                    ptA = p1.tile([128, 1280], mybir.dt.float32,
                                  name="ptA", tag="pt")
                    ptB = p1.tile([128, 1280], mybir.dt.float32,
                                  name="ptB", tag="pt")
                    for ht in range(NT):
                        pt, s = (ptA, ht) if ht < 5 else (ptB, ht - 5)
                        nc.tensor.matmul(
                            pt[:, 256 * s:256 * s + 4 * SLOT],
                            lhsT=xtiles[ht][:, ws:ws + 128],
                            rhs=wt_tiles[ht][:],
                            start=True, stop=True)
                    for f in range(4):
                        t2t = t2p.tile([128, 513], DT,
                                       name=f"t2_{f}_{wt}", tag=f"t2_{f}_{wt}")
                        srcA = ptA.rearrange("p (s c) -> p s c", c=256)[
                            :, :, SLOT * f:SLOT * (f + 1)]
                        dstA = t2t[:, 0:5 * SLOT].rearrange(
                            "p (s c) -> p s c", c=SLOT)
                        srcB = ptB.rearrange("p (s c) -> p s c", c=256)[
                            :, 0:4, SLOT * f:SLOT * (f + 1)]
                        dstB = t2t[:, 5 * SLOT:9 * SLOT].rearrange(
                            "p (s c) -> p s c", c=SLOT)
                        if f % 2 == 0:
                            nc.vector.tensor_copy(dstA, srcA)
                            nc.vector.tensor_copy(dstB, srcB)
                        else:
                            nc.scalar.copy(dstA, srcA)
                            nc.scalar.copy(dstB, srcB)
                        t2[(f, wt)] = t2t

            # ---- pass 2 ----
            with tc.tile_pool(name=f"p2_{i}", bufs=8, space="PSUM") as p2:
                for rb in range(4):
                    ptc = {}
                    for c in CH_ORDER:
                        ptc[c] = p2.tile([128, 512], mybir.dt.float32,
                                         name=f"pc{c}", tag="pc")
                    for wt in range(NT):
                        n0, n1 = assigned(wt)
                        for c in CH_ORDER:
                            ri, ci = CHAN_OPS[c]
                            nc.tensor.matmul(
                                ptc[c][:, n0:n1],
                                lhsT=t2[(ri, wt)][:, 128 * rb:128 * rb + 128],
                                rhs=wt_tiles[wt][:, SLOT * ci:SLOT * ci + (n1 - n0)],
                                start=True, stop=True)
                    for k, c in enumerate(CH_ORDER):
                        ot = outs.tile([128, 512], mybir.dt.float32,
                                       name="ot", tag="ot")
                        if k % 2 == 0:
                            nc.vector.tensor_copy(ot[:], ptc[c][:])
                        else:
                            nc.scalar.copy(ot[:], ptc[c][:])
                        nc.gpsimd.dma_start(
                            y[i, c, 128 * rb:128 * rb + 128, :], ot[:])
    return nc


_CACHED = {}


def _get_nc_and_wmat():
    if "nc" not in _CACHED:
        ops = build_ops()
        wmat = build_bands(ops).astype(NPDT)
        _CACHED["wmat"] = wmat
        nc = build_nc()
        if not nc.is_finalized():
            nc.finalize()
        _CACHED["nc"] = nc
    return _CACHED["nc"], _CACHED["wmat"]


def kernel(x):
    """x: (32, 1, 1024, 1024) float32 -> (32, 6, 512, 512) float32."""
    x = np.ascontiguousarray(np.asarray(x))
    assert x.shape == (B_TOTAL, 1, H, H), x.shape
    nc, wmat = _get_nc_and_wmat()
    in_maps = [
        {"x": np.ascontiguousarray(x[i * BPC:(i + 1) * BPC, 0].astype(np.float32)),
         "w": wmat}
        for i in range(N_CORES)
    ]
    res = run_bass_kernel_spmd(nc, in_maps, list(range(N_CORES)))
    out = np.concatenate([np.asarray(r["y"]) for r in res.results], axis=0)
    return out.astype(np.float32)


# revision 7
# speedup vs baseline: 1.1208x; 1.1208x over previous
"""Trainium2 Bass kernel for nn_DWTExtractor.

Computes, for each single-channel 1024x1024 image, 6 output channels
(3 Haar DWT2 details + 3 Coif1 DWT2 details bilinearly resized to 512x512).

Everything is linear and separable, so each channel is
    chan = RowM @ img @ ColM^T
with RowM/ColM in {Hlo, Hhi, RClo, RChi} (all [512, 1024] banded operators;
RC* fold the coif1 DWT with the jax.image.resize 514->512 linear+antialias
matrix). Both passes run on the TensorEngine with the *data* as the
stationary operand (lhsT), so each pass's output lands in PSUM already
transposed for the next pass - no transpose instructions at all:

  pass 1: T2[op][w, n] = sum_h X[h, w] * Op[n, h]
          lhsT = X[h-window, w-window] (128x128), rhs = packed band matrix
  pass 2: chan[m, n]   = sum_w T2[op][w, m] * Col[n, w]
          lhsT = T2[w-window, m-slice], rhs = band slice

The image axes are covered by 9 overlapping 128-wide windows (stride 114)
so that every output column's 12-tap support lies inside a single window;
each window writes a disjoint column slice (singleton PSUM groups, no
cross-window accumulation). Pass-1 packs all 4 operators' bands into one
[128, 228] rhs per window -> one matmul per (w-window, h-window).

Sharding: pure data parallel, 32 images -> 8 cores x 4 images.
"""

import sys

sys.path.insert(0, "/opt/trn_rl_repo")

from contextlib import ExitStack

import numpy as np

import concourse.bass as bass
from concourse import bacc
import concourse.mybir as mybir
import concourse.tile as tile
from concourse.bass_utils import run_bass_kernel_spmd

# ---------------------------------------------------------------------------
# Host-side operator construction (pure numpy, float64)
# ---------------------------------------------------------------------------

_c = np.array([-0.01565572813546454, -0.0727326195128539, 0.38486484686420286,
               0.8525720202122554, 0.3378976624578092, -0.0727326195128539])
HAAR_LO = np.array([0.7071067811865476, 0.7071067811865476])
HAAR_HI = np.array([-0.7071067811865476, 0.7071067811865476])
COIF1_LO = _c.copy()
COIF1_HI = ((-1.0) ** (np.arange(6) + 1)) * _c[::-1]

H = 1024
NT = 9            # overlapping 128-row windows, stride 114
SLOT = 57         # output columns assigned per window (57 * 9 = 513)
ROW_START = [min(max(114 * t - 6, 0), H - 128) for t in range(NT)]
N_CORES = 8
B_TOTAL = 32
BPC = B_TOTAL // N_CORES

# channel -> (row op index, col op index); ops are [Hlo, Hhi, RClo, RChi]
CHAN_OPS = [(1, 0), (0, 1), (1, 1), (3, 2), (2, 3), (3, 3)]
# channel emission order: channels sharing the same pass-1 tensor adjacent
CH_ORDER = [0, 2, 1, 3, 5, 4]

DT = mybir.dt.float16
NPDT = np.float16


def _dwt1d_np(x, filt):
    L = len(filt)
    n = x.shape[-1]
    xp = np.pad(x, [(0, 0)] * (x.ndim - 1) + [(L - 1, L - 1)], mode="symmetric")
    out_len = (n + L - 1) // 2
    fr = filt[::-1]
    y = np.zeros(x.shape[:-1] + (out_len,), dtype=x.dtype)
    for j in range(L):
        y = y + fr[j] * xp[..., 1 + j:1 + j + 2 * out_len:2]
    return y


def _dwt_matrix(n, filt):
    eye = np.eye(n, dtype=np.float64)
    return _dwt1d_np(eye, np.asarray(filt, np.float64)).T.copy()


def _resize_matrix(in_size, out_size):
    """Replicates jax.image.resize(method='linear', antialias=True)."""
    scale = out_size / in_size
    inv_scale = 1.0 / scale
    kernel_scale = max(inv_scale, 1.0)
    sample_f = (np.arange(out_size, dtype=np.float64) + 0.5) * inv_scale - 0.5
    x = np.abs(sample_f[None, :]
               - np.arange(in_size, dtype=np.float64)[:, None]) / kernel_scale
    w = np.maximum(0.0, 1.0 - x)
    total = w.sum(axis=0, keepdims=True)
    w = np.where(np.abs(total) > 1000.0 * np.finfo(np.float32).eps,
                 w / np.where(total != 0, total, 1), 0.0)
    w = np.where(((sample_f >= -0.5) & (sample_f <= in_size - 0.5))[None, :],
                 w, 0.0)
    return w.T.copy()


def build_ops():
    Hlo = _dwt_matrix(H, HAAR_LO)
    Hhi = _dwt_matrix(H, HAAR_HI)
    Clo = _dwt_matrix(H, COIF1_LO)
    Chi = _dwt_matrix(H, COIF1_HI)
    R = _resize_matrix(514, 512)
    return [Hlo, Hhi, R @ Clo, R @ Chi]


def assigned(t):
    return SLOT * t, min(SLOT * (t + 1), 512)


def build_bands(ops):
    """wmat [NT, 128, 4*SLOT]: per-window packed band matrices."""
    wmat = np.zeros((NT, 128, 4 * SLOT), np.float64)
    for t in range(NT):
        rs = ROW_START[t]
        n0, n1 = assigned(t)
        for f in range(4):
            full = ops[f][n0:n1]
            mask = np.zeros(H, bool)
            mask[rs:rs + 128] = True
            assert np.abs(full[:, ~mask]).max() == 0.0, (t, f)
            wmat[t, :, f * SLOT:f * SLOT + (n1 - n0)] = full[:, rs:rs + 128].T
    return wmat


# ---------------------------------------------------------------------------
# Bass kernel
# ---------------------------------------------------------------------------

def build_nc(bpc=BPC):
    nc = bacc.Bacc("TRN2", num_swdge_queues=4)
    x = nc.dram_tensor("x", [bpc, H, H], mybir.dt.float32, kind="ExternalInput")
    w = nc.dram_tensor("w", [NT, 128, 4 * SLOT], DT, kind="ExternalInput")
    y = nc.dram_tensor("y", [bpc, 6, 512, 512], mybir.dt.float32,
                       kind="ExternalOutput")

    with tile.TileContext(nc) as tc, ExitStack() as ctx:
        const = ctx.enter_context(tc.tile_pool(name="const", bufs=1))
        xstage = ctx.enter_context(tc.tile_pool(name="xstage", bufs=3))
        xhalf = ctx.enter_context(tc.tile_pool(name="xhalf", bufs=2))
        t2p = ctx.enter_context(tc.tile_pool(name="t2p", bufs=2))
        outs = ctx.enter_context(tc.tile_pool(name="outs", bufs=8))

        wt_tiles = []
        for t in range(NT):
            wt_t = const.tile([128, 4 * SLOT], DT, name=f"w{t}", tag=f"w{t}")
            nc.sync.dma_start(wt_t[:], w[t])
            wt_tiles.append(wt_t)

        for i in range(bpc):
            # ---- load + fp16 cast ----
            xtiles = []
            for t in range(NT):
                xs_t = xstage.tile([128, H], mybir.dt.float32,
                                   name="xs", tag="xs")
                nc.sync.dma_start(
                    xs_t[:], x[i, ROW_START[t]:ROW_START[t] + 128, :])
                xb_t = xhalf.tile([128, H], DT, name=f"xb{t}", tag=f"xb{t}")
                cast_eng = [nc.gpsimd, nc.vector, nc.scalar][t % 3]
                if t % 3 == 2:
                    cast_eng.copy(xb_t[:], xs_t[:])
                else:
                    cast_eng.tensor_copy(xb_t[:], xs_t[:])
                xtiles.append(xb_t)

            # ---- pass 1 ----
            t2 = {}
            with tc.tile_pool(name=f"p1_{i}", bufs=2, space="PSUM") as p1:
                for wt in range(NT):
                    ws = ROW_START[wt]
                    ptA = p1.tile([128, 1280], mybir.dt.float32,
                                  name="ptA", tag="pt")
                    ptB = p1.tile([128, 1280], mybir.dt.float32,
                                  name="ptB", tag="pt")
                    for ht in range(NT):
                        pt, s = (ptA, ht) if ht < 5 else (ptB, ht - 5)
                        nc.tensor.matmul(
                            pt[:, 256 * s:256 * s + 4 * SLOT],
                            lhsT=xtiles[ht][:, ws:ws + 128],
                            rhs=wt_tiles[ht][:],
                            start=True, stop=True)
                    for f in range(4):
                        t2t = t2p.tile([128, 513], DT,
                                       name=f"t2_{f}_{wt}", tag=f"t2_{f}_{wt}")
                        srcA = ptA.rearrange("p (s c) -> p s c", c=256)[
                            :, :, SLOT * f:SLOT * (f + 1)]
                        dstA = t2t[:, 0:5 * SLOT].rearrange(
                            "p (s c) -> p s c", c=SLOT)
                        srcB = ptB.rearrange("p (s c) -> p s c", c=256)[
                            :, 0:4, SLOT * f:SLOT * (f + 1)]
                        dstB = t2t[:, 5 * SLOT:9 * SLOT].rearrange(
                            "p (s c) -> p s c", c=SLOT)
                        if f % 2 == 0:
                            nc.vector.tensor_copy(dstA, srcA)
                            nc.vector.tensor_copy(dstB, srcB)
                        else:
                            nc.scalar.copy(dstA, srcA)
                            nc.scalar.copy(dstB, srcB)
                        t2[(f, wt)] = t2t

            # ---- pass 2 ----
            with tc.tile_pool(name=f"p2_{i}", bufs=8, space="PSUM") as p2:
                for rb in range(4):
                    ptc = {}
                    for c in CH_ORDER:
                        ptc[c] = p2.tile([128, 512], mybir.dt.float32,
                                         name=f"pc{c}", tag="pc")
                    for wt in range(NT):
                        n0, n1 = assigned(wt)
                        for c in CH_ORDER:
                            ri, ci = CHAN_OPS[c]
                            nc.tensor.matmul(
                                ptc[c][:, n0:n1],
                                lhsT=t2[(ri, wt)][:, 128 * rb:128 * rb + 128],
                                rhs=wt_tiles[wt][:, SLOT * ci:SLOT * ci + (n1 - n0)],
                                start=True, stop=True)
                    for k, c in enumerate(CH_ORDER):
                        ot = outs.tile([128, 512], mybir.dt.float32,
                                       name="ot", tag="ot")
                        if k % 2 == 0:
                            nc.vector.tensor_copy(ot[:], ptc[c][:])
                        else:
                            nc.scalar.copy(ot[:], ptc[c][:])
                        nc.sync.dma_start(
                            y[i, c, 128 * rb:128 * rb + 128, :], ot[:])
    return nc


_CACHED = {}


def _get_nc_and_wmat():
    if "nc" not in _CACHED:
        ops = build_ops()
        wmat = build_bands(ops).astype(NPDT)
        _CACHED["wmat"] = wmat
        nc = build_nc()
        if not nc.is_finalized():
            nc.finalize()
        _CACHED["nc"] = nc
    return _CACHED["nc"], _CACHED["wmat"]


def kernel(x):
    """x: (32, 1, 1024, 1024) float32 -> (32, 6, 512, 512) float32."""
    x = np.ascontiguousarray(np.asarray(x))
    assert x.shape == (B_TOTAL, 1, H, H), x.shape
    nc, wmat = _get_nc_and_wmat()
    in_maps = [
        {"x": np.ascontiguousarray(x[i * BPC:(i + 1) * BPC, 0].astype(np.float32)),
         "w": wmat}
        for i in range(N_CORES)
    ]
    res = run_bass_kernel_spmd(nc, in_maps, list(range(N_CORES)))
    out = np.concatenate([np.asarray(r["y"]) for r in res.results], axis=0)
    return out.astype(np.float32)


# revision 13
# speedup vs baseline: 1.4946x; 1.3335x over previous
"""Trainium2 Bass kernel for nn_DWTExtractor.

Computes, for each single-channel 1024x1024 image, 6 output channels
(3 Haar DWT2 details + 3 Coif1 DWT2 details bilinearly resized to 512x512).

Everything is linear and separable, so each channel is
    chan = RowM @ img @ ColM^T
with RowM/ColM in {Hlo, Hhi, RClo, RChi} (all [512, 1024] banded operators;
RC* fold the coif1 DWT with the jax.image.resize 514->512 linear+antialias
matrix). Both passes run on the TensorEngine with the *data* as the
stationary operand (lhsT), so each pass's output lands in PSUM already
transposed for the next pass - no transpose instructions at all:

  pass 1: T2[op][w, n] = sum_h X[h, w] * Op[n, h]
          lhsT = X[h-window, w-window] (128x128), rhs = packed band matrix
  pass 2: chan[m, n]   = sum_w T2[op][w, m] * Col[n, w]
          lhsT = T2[w-window, m-slice], rhs = band slice

The image axes are covered by 9 overlapping 128-wide windows (stride 114)
so that every output column's 12-tap support lies inside a single window;
each window writes a disjoint column slice (singleton PSUM groups, no
cross-window accumulation). Pass-1 packs all 4 operators' bands into one
[128, 228] rhs per window -> one matmul per (w-window, h-window).

Sharding: pure data parallel, 32 images -> 8 cores x 4 images.
"""

import sys

sys.path.insert(0, "/opt/trn_rl_repo")

from contextlib import ExitStack

import numpy as np

import concourse.bass as bass
from concourse import bacc
import concourse.mybir as mybir
import concourse.tile as tile
from concourse.bass_utils import run_bass_kernel_spmd

# ---------------------------------------------------------------------------
# Host-side operator construction (pure numpy, float64)
# ---------------------------------------------------------------------------

_c = np.array([-0.01565572813546454, -0.0727326195128539, 0.38486484686420286,
               0.8525720202122554, 0.3378976624578092, -0.0727326195128539])
HAAR_LO = np.array([0.7071067811865476, 0.7071067811865476])
HAAR_HI = np.array([-0.7071067811865476, 0.7071067811865476])
COIF1_LO = _c.copy()
COIF1_HI = ((-1.0) ** (np.arange(6) + 1)) * _c[::-1]

H = 1024
NT = 9            # overlapping 128-row windows, stride 114
SLOT = 57         # output columns assigned per window (57 * 9 = 513)
ROW_START = [min(max(114 * t - 6, 0), H - 128) for t in range(NT)]
N_CORES = 8
B_TOTAL = 32
BPC = B_TOTAL // N_CORES

# channel -> (row op index, col op index); ops are [Hlo, Hhi, RClo, RChi]
CHAN_OPS = [(1, 0), (0, 1), (1, 1), (3, 2), (2, 3), (3, 3)]
# channel emission order: channels sharing the same pass-1 tensor adjacent
CH_ORDER = [0, 2, 1, 3, 5, 4]

DT = mybir.dt.float16
NPDT = np.float16


def _dwt1d_np(x, filt):
    L = len(filt)
    n = x.shape[-1]
    xp = np.pad(x, [(0, 0)] * (x.ndim - 1) + [(L - 1, L - 1)], mode="symmetric")
    out_len = (n + L - 1) // 2
    fr = filt[::-1]
    y = np.zeros(x.shape[:-1] + (out_len,), dtype=x.dtype)
    for j in range(L):
        y = y + fr[j] * xp[..., 1 + j:1 + j + 2 * out_len:2]
    return y


def _dwt_matrix(n, filt):
    eye = np.eye(n, dtype=np.float64)
    return _dwt1d_np(eye, np.asarray(filt, np.float64)).T.copy()


def _resize_matrix(in_size, out_size):
    """Replicates jax.image.resize(method='linear', antialias=True)."""
    scale = out_size / in_size
    inv_scale = 1.0 / scale
    kernel_scale = max(inv_scale, 1.0)
    sample_f = (np.arange(out_size, dtype=np.float64) + 0.5) * inv_scale - 0.5
    x = np.abs(sample_f[None, :]
               - np.arange(in_size, dtype=np.float64)[:, None]) / kernel_scale
    w = np.maximum(0.0, 1.0 - x)
    total = w.sum(axis=0, keepdims=True)
    w = np.where(np.abs(total) > 1000.0 * np.finfo(np.float32).eps,
                 w / np.where(total != 0, total, 1), 0.0)
    w = np.where(((sample_f >= -0.5) & (sample_f <= in_size - 0.5))[None, :],
                 w, 0.0)
    return w.T.copy()


def build_ops():
    Hlo = _dwt_matrix(H, HAAR_LO)
    Hhi = _dwt_matrix(H, HAAR_HI)
    Clo = _dwt_matrix(H, COIF1_LO)
    Chi = _dwt_matrix(H, COIF1_HI)
    R = _resize_matrix(514, 512)
    return [Hlo, Hhi, R @ Clo, R @ Chi]


def assigned(t):
    return SLOT * t, min(SLOT * (t + 1), 512)


def build_bands(ops):
    """wmat [NT, 128, 4*SLOT]: per-window packed band matrices."""
    wmat = np.zeros((NT, 128, 4 * SLOT), np.float64)
    for t in range(NT):
        rs = ROW_START[t]
        n0, n1 = assigned(t)
        for f in range(4):
            full = ops[f][n0:n1]
            mask = np.zeros(H, bool)
            mask[rs:rs + 128] = True
            assert np.abs(full[:, ~mask]).max() == 0.0, (t, f)
            wmat[t, :, f * SLOT:f * SLOT + (n1 - n0)] = full[:, rs:rs + 128].T
    return wmat


# ---------------------------------------------------------------------------
# Bass kernel
# ---------------------------------------------------------------------------

def build_nc(bpc=BPC):
    nc = bacc.Bacc("TRN2", num_swdge_queues=4)
    x = nc.dram_tensor("x", [bpc, H, H], mybir.dt.float32, kind="ExternalInput")
    w = nc.dram_tensor("w", [NT, 128, 4 * SLOT], DT, kind="ExternalInput")
    y = nc.dram_tensor("y", [bpc, 6, 512, 512], mybir.dt.float32,
                       kind="ExternalOutput")

    with tile.TileContext(nc) as tc, ExitStack() as ctx:
        const = ctx.enter_context(tc.tile_pool(name="const", bufs=1))
        xstage = ctx.enter_context(tc.tile_pool(name="xstage", bufs=3))
        xhalf = ctx.enter_context(tc.tile_pool(name="xhalf", bufs=2))
        t2p = ctx.enter_context(tc.tile_pool(name="t2p", bufs=2))
        outs = ctx.enter_context(tc.tile_pool(name="outs", bufs=8))
        psum = ctx.enter_context(tc.tile_pool(name="psum", bufs=1,
                                              space="PSUM"))

        wt_tiles = []
        for t in range(NT):
            wt_t = const.tile([128, 4 * SLOT], DT, name=f"w{t}", tag=f"w{t}")
            nc.sync.dma_start(wt_t[:], w[t])
            wt_tiles.append(wt_t)

        for i in range(bpc):
            # ---- load + fp16 cast ----
            xtiles = []
            for t in range(NT):
                xs_t = xstage.tile([128, H], mybir.dt.float32,
                                   name="xs", tag="xs")
                nc.sync.dma_start(
                    xs_t[:], x[i, ROW_START[t]:ROW_START[t] + 128, :])
                xb_t = xhalf.tile([128, H], DT, name=f"xb{t}", tag=f"xb{t}")
                cast_eng = [nc.gpsimd, nc.vector, nc.scalar][t % 3]
                if t % 3 == 2:
                    cast_eng.copy(xb_t[:], xs_t[:])
                else:
                    cast_eng.tensor_copy(xb_t[:], xs_t[:])
                xtiles.append(xb_t)

            # ---- pass 1 ----
            # psum slot layout: slot s (of 9) at col 256*s, 228 used cols
            # (4 ops x 57). ptA holds slots 0-4, ptB slots 5-8. T2pack keeps
            # the same packed layout in fp16; pass-2 slices it per op.
            t2 = {}
            for wt in range(NT):
                ws = ROW_START[wt]
                ptA = psum.tile([128, 1280], mybir.dt.float32,
                                name="ptA", tag="ptA")
                ptB = psum.tile([128, 1024], mybir.dt.float32,
                                name="ptB", tag="ptB")
                for ht in range(NT):
                    pt, s = (ptA, ht) if ht < 5 else (ptB, ht - 5)
                    nc.tensor.matmul(
                        pt[:, 256 * s:256 * s + 4 * SLOT],
                        lhsT=xtiles[ht][:, ws:ws + 128],
                        rhs=wt_tiles[ht][:],
                        start=True, stop=True)
                # T2 layout is op-major: op f occupies cols [513f, 513f+513),
                # so pass-2 lhsT slices are single-free-dim. The copies
                # de-interleave the psum slot layout via 3-free-dim APs.
                t2t = t2p.tile([128, 4 * 513], DT,
                               name=f"t2_{wt}", tag=f"t2_{wt}")
                t2r = t2t.rearrange("p (f s j) -> p s f j", f=4, s=NT, j=SLOT)
                srcA = ptA.rearrange("p (s c) -> p s c", c=256)[
                    :, :, 0:228].rearrange("p s (f j) -> p s f j", j=SLOT)
                srcB = ptB.rearrange("p (s c) -> p s c", c=256)[
                    :, :, 0:228].rearrange("p s (f j) -> p s f j", j=SLOT)
                nc.vector.tensor_copy(t2r[:, 0:5], srcA)
                nc.scalar.copy(t2r[:, 5:NT], srcB)
                t2[wt] = t2t

            # ---- pass 2 ----
            for rb in range(4):
                for group in ((0, 2), (1,), (3, 5), (4,)):
                    ptc = {}
                    for c in group:
                        ptc[c] = psum.tile([128, 512], mybir.dt.float32,
                                           name=f"pc{c}", tag="pc", bufs=3)
                    for wt in range(NT):
                        n0, n1 = assigned(wt)
                        for c in group:
                            ri, ci = CHAN_OPS[c]
                            nc.tensor.matmul(
                                ptc[c][:, n0:n1],
                                lhsT=t2[wt][:, 513 * ri + 128 * rb:
                                            513 * ri + 128 * rb + 128],
                                rhs=wt_tiles[wt][:, SLOT * ci:SLOT * ci + (n1 - n0)],
                                start=True, stop=True)
                    for k, c in enumerate(group):
                        ot = outs.tile([128, 512], mybir.dt.float32,
                                       name="ot", tag="ot")
                        if (rb + k) % 2 == 0:
                            nc.vector.tensor_copy(ot[:], ptc[c][:])
                        else:
                            nc.scalar.copy(ot[:], ptc[c][:])
                        nc.sync.dma_start(
                            y[i, c, 128 * rb:128 * rb + 128, :], ot[:])
    return nc


_CACHED = {}


def _get_nc_and_wmat():
    if "nc" not in _CACHED:
        ops = build_ops()
        wmat = build_bands(ops).astype(NPDT)
        _CACHED["wmat"] = wmat
        nc = build_nc()
        if not nc.is_finalized():
            nc.finalize()
        _CACHED["nc"] = nc
    return _CACHED["nc"], _CACHED["wmat"]


def kernel(x):
    """x: (32, 1, 1024, 1024) float32 -> (32, 6, 512, 512) float32."""
    x = np.ascontiguousarray(np.asarray(x))
    assert x.shape == (B_TOTAL, 1, H, H), x.shape
    nc, wmat = _get_nc_and_wmat()
    in_maps = [
        {"x": np.ascontiguousarray(x[i * BPC:(i + 1) * BPC, 0].astype(np.float32)),
         "w": wmat}
        for i in range(N_CORES)
    ]
    res = run_bass_kernel_spmd(nc, in_maps, list(range(N_CORES)))
    out = np.concatenate([np.asarray(r["y"]) for r in res.results], axis=0)
    return out.astype(np.float32)
